# revision 53
# baseline (speedup 1.0000x reference)
"""KMeans vq_codebook kernel for 8 trn2 NeuronCores.

Data-parallel over N (per sharding hint), 32768 rows/core, 256 row-tiles of
128. Per 4-tile PSUM batch:

  PE:   bias matmuls: g2 = -csq broadcast (fp16 [2 x 512]-col, PSUM init)
        main matmuls: g2[:,q,:] += xh_tile^T @ ch  (ONE fp16 matmul per tile;
        host splits x to fp16 - dropped lo terms cost ~3e-4 rel acc)
        conf matmuls: fp8 DoubleRow, contracting TWO tiles per instruction
        (yoh padded to 16 classes; indicator values exact in fp8)
  DVE:  hmax = tensor_reduce(max) over the fp32 PSUM batch (exact)
  Indicator (by batch, balancing DVE vs ACT):
    - 7 of 8 batches on ACT: sign(g2 - hmax + DELTA) per tile (per-partition
      bias AP), values {-1,+1} -> confb; host linearizes A=(confb+n_c)/2
    - 1 of 8 batches on DVE: is_ge(g2, hmax) batched (exact) -> confa

  loss = sum(x^2) (host fp64) - sum(hmax);  acc from conf on host.
"""

import numpy as np

try:
    import concourse.bass as bass
    import concourse.mybir as mybir
    import concourse.tile as tile
    from concourse.bass_utils import run_bass_kernel_spmd
except ImportError:  # allow sys.path setup by the caller
    import sys

    sys.path.insert(0, "/opt/trn_rl_repo")
    import concourse.bass as bass
    import concourse.mybir as mybir
    import concourse.tile as tile
    from concourse.bass_utils import run_bass_kernel_spmd

N_FULL = 262144
D = 128
K = 256
NUM_CORES = 8
NS = N_FULL // NUM_CORES  # 32768 rows per core
NTILES = NS // 128  # 256
NUM_GT_CLASSES = 10
NGC = 16  # padded to 16 for fp8 DoubleRow conf matmuls (rows 10..15 zero)

BATCH = 4  # tiles per PSUM batch
BATCH_PERIOD = 7  # every BATCH_PERIOD-th batch runs its indicator on DVE.
# HW-measured optimum: bp=4 -> 245us, bp=7 -> 171us, all-ACT -> 192us; the
# cost model is near-flat here, so trust the hardware A/B.
DELTA = 2e-3  # ACT sign margin: sign(h - hmax + DELTA) is +1 at the argmax

F32 = mybir.dt.float32
F16 = mybir.dt.float16
F8 = mybir.dt.float8e4

_CACHE = {}
LAST = None  # BassKernelResults of the most recent run
HW_EXEC_NS = None  # per-execution device time measured by _bench_exec


def _is_dve_batch(b):
    return (b % BATCH_PERIOD) == (BATCH_PERIOD - 1)


def _is_act_tile(j):
    return not _is_dve_batch(j // BATCH)


LAG_OVERRIDE = 2
OH_BUFS = 5


def build_nc(ns=NS, supertile=2048, batch=BATCH, g2bufs=3, repeats=1):
    """repeats>1 builds a benchmarking variant: the whole per-core program
    body is repeated in-NEFF (identical results, overwritten) so device time
    dominates per-dispatch overhead when measuring."""
    ntiles = ns // 128
    tiles_per_super = supertile // 128
    nbatch = ntiles // batch
    assert tiles_per_super % batch == 0
    assert batch % 2 == 0

    import concourse.bacc as bacc

    nc = bacc.Bacc("TRN2", target_bir_lowering=False, debug=False)

    xh_d = nc.declare_dram_parameter("xh", [D, ns], F16, isOutput=False)
    ch_d = nc.declare_dram_parameter("ch", [D, K], F16, isOutput=False)
    # [2, batch*K] fp16: rows = csq hi/lo, tiled `batch` times, negated
    ncsq_d = nc.declare_dram_parameter("ncsq", [2, batch * K], F16, isOutput=False)
    yoh_d = nc.declare_dram_parameter("yoh", [128, ntiles, NGC], F8, isOutput=False)
    yoh16_d = nc.declare_dram_parameter(
        "yoh16", [128, ntiles, NGC], F16, isOutput=False
    )
    hst_out = nc.declare_dram_parameter("hst", [128, ntiles], F32, isOutput=True)
    confa_out = nc.declare_dram_parameter("confa", [NGC, K], F32, isOutput=True)
    confb_out = nc.declare_dram_parameter("confb", [NGC, K], F32, isOutput=True)

    dve_batches = [b for b in range(nbatch) if _is_dve_batch(b)]
    dve_pairs = [
        (b, p) for b in range(nbatch) for p in range(batch // 2) if _is_dve_batch(b)
    ]
    act_pairs = [
        (b, p)
        for b in range(nbatch)
        for p in range(batch // 2)
        if not _is_dve_batch(b)
    ]
    LAG = LAG_OVERRIDE  # conf matmuls emitted after fill of batch b+LAG

    with tile.TileContext(nc) as tc:
        with (
            tc.tile_pool(name="const", bufs=1) as constp,
            tc.tile_pool(name="xs", bufs=3) as xsp,
            tc.tile_pool(name="oh", bufs=OH_BUFS) as ohp,
            tc.tile_pool(name="hn", bufs=3) as hnp,
            tc.tile_pool(name="acc", bufs=1) as accp,
            tc.tile_pool(name="ps", bufs=g2bufs, space=bass.MemorySpace.PSUM) as psp,
            tc.tile_pool(name="psca", bufs=1, space=bass.MemorySpace.PSUM) as pscap,
            tc.tile_pool(name="pscb", bufs=1, space=bass.MemorySpace.PSUM) as pscbp,
        ):
            ch_t = constp.tile([D, K], F16, tag="ch")
            ones2_t = constp.tile([2, 128], F16, tag="ones2")
            ncsq_t = constp.tile([2, batch * K], F16, tag="ncsq")
            yoh_t = constp.tile([128, ntiles, NGC], F8, tag="yoh")
            yoh16_t = constp.tile([128, ntiles, NGC], F16, tag="yoh16")
            nc.sync.dma_start(ch_t[:], ch_d[:, :])
            nc.sync.dma_start(ncsq_t[:], ncsq_d[:, :])
            nc.vector.memset(ones2_t[:], 1.0)

            hstore = accp.tile([128, ntiles], F32, tag="hst")
            confa_ps = pscap.tile([NGC, K], F32, tag="confa")
            confb_ps = pscbp.tile([NGC, K], F32, tag="confb")

            xh_tiles = {}
            n_super = ns // supertile
            PF = 2  # supertiles of x prefetched beyond the one in use

            def issue_xh(st):
                if st in xh_tiles or st >= n_super:
                    return
                xh_s = xsp.tile([D, supertile], F16, tag="xh")
                nc.sync.dma_start(
                    xh_s[:], xh_d[:, st * supertile : (st + 1) * supertile]
                )
                xh_tiles[st] = xh_s

            def emit_fill(b):
                st = b // (tiles_per_super // batch)
                bb = b % (tiles_per_super // batch)
                if bb == 0:
                    issue_xh(st + PF)
                xh_s = xh_tiles[st]
                g2 = psp.tile([128, batch, K], F32, tag="g2")
                g2flat = g2[:].rearrange("p a b -> p (a b)")
                half = batch * K // 2  # 512 fp32 = one PSUM bank
                for hh in range(2):
                    nc.tensor.matmul(
                        g2flat[:, hh * half : (hh + 1) * half],
                        ones2_t[:],
                        ncsq_t[:, hh * half : (hh + 1) * half],
                        start=True,
                        stop=False,
                        skip_group_check=True,
                    )
                for q in range(batch):
                    sl = slice((bb * batch + q) * 128, (bb * batch + q + 1) * 128)
                    nc.tensor.matmul(
                        g2[:, q, :],
                        xh_s[:, sl],
                        ch_t[:],
                        start=False,
                        stop=True,
                        skip_group_check=True,
                    )
                return g2

            def emit_reduce(b, g2):
                """DVE TR-max. DVE batches store +hmax; ACT batches store
                -hmax (used directly as the ACT sign bias; host negates)."""
                jb = b * batch
                nc.vector.tensor_reduce(
                    hstore[:, jb : jb + batch],
                    g2[:],
                    axis=mybir.AxisListType.X,
                    op=mybir.AluOpType.max,
                    negate=not _is_dve_batch(b),
                )

            def emit_signs(b, g2, oh4):
                jb = b * batch
                for q in range(batch):
                    j = jb + q
                    nc.scalar.activation(
                        oh4[:, q, :],
                        g2[:, q, :],
                        mybir.ActivationFunctionType.Sign,
                        bias=hstore[:, j : j + 1],
                        scale=1.0,
                    )

            def emit_iseq(b, g2, oh4):
                """Batched exact compare on DVE (deferred one step so the next
                batch's TR is never stuck behind it)."""
                jb = b * batch
                nc.vector.tensor_tensor(
                    oh4[:],
                    g2[:],
                    hstore[:, jb : jb + batch]
                    .unsqueeze(2)
                    .broadcast_to([128, batch, K]),
                    mybir.AluOpType.is_ge,
                )

            def emit_conf(b, oh4):
                jb = b * batch
                if _is_dve_batch(b):
                    # fp16 oh (fp8 DVE output is slow on HW): plain matmuls
                    for q in range(batch):
                        j = jb + q
                        nc.tensor.matmul(
                            confa_ps[:],
                            yoh16_t[:, j, :],
                            oh4[:, q, :],
                            start=(b == dve_batches[0] and q == 0),
                            stop=(b == dve_batches[-1] and q == batch - 1),
                            skip_group_check=True,
                        )
                    return
                for p in range(batch // 2):
                    j0 = jb + 2 * p
                    nc.tensor.matmul(
                        confb_ps[:],
                        yoh_t[:, j0 : j0 + 2, :],
                        oh4[:, 2 * p : 2 * p + 2, :],
                        start=((b, p) == act_pairs[0]),
                        stop=((b, p) == act_pairs[-1]),
                        skip_group_check=True,
                        perf_mode=mybir.MatmulPerfMode.DoubleRow,
                    )

            oh_tiles = {}
            for _rep in range(repeats):
                xh_tiles.clear()
                for s in range(min(PF + 1, n_super)):
                    issue_xh(s)
                if _rep == 0:
                    # yoh is first needed by the conf stage ~3 batches in;
                    # keep it off the critical startup DMA path
                    nc.sync.dma_start(yoh_t[:], yoh_d[:, :, :])
                    nc.sync.dma_start(yoh16_t[:], yoh16_d[:, :, :])
                g2_tiles = {}
                for t in range(nbatch + LAG + 1):
                    if t < nbatch:
                        g2_tiles[t] = emit_fill(t)
                    b = t - 1
                    if 0 <= b < nbatch:
                        emit_reduce(b, g2_tiles[b])
                        oh4 = ohp.tile(
                            [128, batch, K],
                            F16 if _is_dve_batch(b) else F8,
                            tag="oh4a" if _is_dve_batch(b) else "oh4",
                        )
                        oh_tiles[b] = oh4
                        if not _is_dve_batch(b):
                            emit_signs(b, g2_tiles.pop(b), oh4)
                    b = t - 2
                    if 0 <= b < nbatch and _is_dve_batch(b):
                        emit_iseq(b, g2_tiles.pop(b), oh_tiles[b])
                    b = t - LAG - 1
                    if 0 <= b < nbatch:
                        emit_conf(b, oh_tiles.pop(b))

            confa_sb = accp.tile([NGC, K], F32, tag="confasb")
            confb_sb = accp.tile([NGC, K], F32, tag="confbsb")
            if dve_pairs:
                nc.vector.tensor_copy(confa_sb[:], confa_ps[:])
            else:
                nc.vector.memset(confa_sb[:], 0.0)
            if act_pairs:
                nc.vector.tensor_copy(confb_sb[:], confb_ps[:])
            else:
                nc.vector.memset(confb_sb[:], 0.0)
            nc.sync.dma_start(hst_out[:, :], hstore[:])
            nc.sync.dma_start(confa_out[:, :], confa_sb[:])
            nc.sync.dma_start(confb_out[:, :], confb_sb[:])

    nc.compile()
    return nc


def _host_prep(x, y_np, centers):
    """Build per-core input maps from full inputs."""
    import ml_dtypes

    f8 = ml_dtypes.float8_e4m3fn
    xt = np.ascontiguousarray(x.T).astype(np.float16)  # [128, N]
    c2t = np.ascontiguousarray(centers.T) * np.float32(2.0)
    ch = c2t.astype(np.float16)  # [128, K]

    csq = np.sum(centers.astype(np.float64) ** 2, axis=1)
    ncsq_hi = (-csq).astype(np.float16)
    ncsq_lo = ((-csq) - ncsq_hi.astype(np.float64)).astype(np.float16)
    ncsq = np.empty((2, BATCH * K), np.float16)
    ncsq[0] = np.tile(ncsq_hi, BATCH)
    ncsq[1] = np.tile(ncsq_lo, BATCH)

    y_cores = y_np.reshape(NUM_CORES, NTILES, 128)  # [core, tile, p]
    oh = (y_cores[:, :, :, None] == np.arange(NGC)[None, None, None, :]).astype(f8)
    yoh_all = np.ascontiguousarray(oh.transpose(0, 2, 1, 3))  # [core,128,tile,16]

    in_maps = []
    for c in range(NUM_CORES):
        sl = slice(c * NS, (c + 1) * NS)
        in_maps.append(
            {
                "xh": np.ascontiguousarray(xt[:, sl]),
                "ch": ch,
                "ncsq": ncsq,
                "yoh": yoh_all[c],
                "yoh16": yoh_all[c].astype(np.float16),
            }
        )
    return in_maps


def kernel(x, y, centers):
    x = np.asarray(x, dtype=np.float32)
    y_np = np.asarray(y).astype(np.int64)
    centers = np.asarray(centers, dtype=np.float32)
    n = x.shape[0]
    assert n == N_FULL and x.shape[1] == D and centers.shape == (K, D)

    if "nc" not in _CACHE:
        _CACHE["nc"] = build_nc()
    nc = _CACHE["nc"]

    in_maps = _host_prep(x, y_np, centers)

    kr = run_bass_kernel_spmd(nc, in_maps, list(range(NUM_CORES)))
    global LAST, HW_EXEC_NS
    LAST = kr
    res = kr.results

    import os

    if os.environ.get("BASS_BENCH") == "1":
        reps = int(os.environ.get("BASS_BENCH_REPS", "16"))
        if reps > 1:
            if "nc_bench" not in _CACHE:
                _CACHE["nc_bench"] = build_nc(repeats=reps)
            HW_EXEC_NS = _bench_exec(_CACHE["nc_bench"], in_maps, NUM_CORES) // reps
        else:
            HW_EXEC_NS = _bench_exec(nc, in_maps, NUM_CORES)

    # ACT-routed tiles store (DELTA - hmax) and count into confb with sign
    # values {-1,+1} (linearized below); DVE tiles store +hmax -> confa.
    act_tile = np.array([_is_act_tile(j) for j in range(NTILES)])

    hmax_sum = 0.0
    confa = np.zeros((NGC, K), np.float64)
    confb = np.zeros((NGC, K), np.float64)
    for c in range(NUM_CORES):
        hst = np.asarray(res[c]["hst"]).astype(np.float64)  # [128, ntiles]
        hmax = np.where(act_tile[None, :], -hst, hst)
        hmax_sum += hmax.sum()
        confa += np.asarray(res[c]["confa"]).astype(np.float64)
        confb += np.asarray(res[c]["confb"]).astype(np.float64)

    # per-class sample counts within ACT tiles (same tile pattern per core)
    y_tiles = y_np.reshape(NUM_CORES * NTILES, 128)
    act_mask = np.tile(act_tile, NUM_CORES)
    n_act = np.bincount(y_tiles[act_mask].ravel(), minlength=NGC).astype(np.float64)

    # ACT path computed sign(h - hmax): argmax element -> sign(0) = s0 (HW
    # table value, 0 or 1), others -> -1. confb[c,k] = (s0+1)*A - n_c, so
    # A = (confb + n_c) / (s0 + 1). Calibrate s0 from the column sums:
    # sum_k confb[c,:] = (s0+1)*n_c - K*n_c.
    nz = n_act > 0
    s0_est = (confb.sum(axis=1)[nz] / n_act[nz]).mean() + K - 1
    s0 = int(round(s0_est))
    assert s0 in (0, 1), f"unexpected HW sign(0) semantics: s0={s0_est}"
    conf = confa + (confb + n_act[:, None]) / (s0 + 1.0)  # [16, K]

    x64 = x.astype(np.float64)
    x_sq_total = float(np.einsum("nd,nd->", x64, x64, optimize=True))
    loss = np.float32(x_sq_total - hmax_sum)

    correct_ct = conf[:NUM_GT_CLASSES].max(axis=0).sum()
    acc = np.float32(correct_ct / np.float64(n))
    return loss, acc


def _bench_exec(nc, in_maps, n_cores):
    """Estimate per-execution device time of the compiled NEFF.

    Replicates bass2jax.run_bass_via_pjrt's jit(shard_map(custom_call))
    plumbing, but keeps the jitted callable and times pipelined repeated
    executions, reporting the marginal time per execution.
    """
    import time

    import jax
    from jax.experimental.shard_map import shard_map
    from jax.sharding import Mesh, NamedSharding, PartitionSpec

    from concourse import bass2jax as b2j

    b2j.install_neuronx_cc_hook()
    partition_name = nc.partition_id_tensor.name if nc.partition_id_tensor else None
    in_names, out_names, out_avals, zero_outs = [], [], [], []
    for alloc in nc.m.functions[0].allocations:
        if not isinstance(alloc, mybir.MemoryLocationSet):
            continue
        name = alloc.memorylocations[0].name
        if alloc.kind == "ExternalInput":
            if name != partition_name:
                in_names.append(name)
        elif alloc.kind == "ExternalOutput":
            out_names.append(name)
            shape = tuple(alloc.tensor_shape)
            dtype = mybir.dt.np(alloc.dtype)
            out_avals.append(jax.core.ShapedArray(shape, dtype))
            zero_outs.append(np.zeros(shape, dtype))
    n_params = len(in_names)
    n_outs = len(out_avals)
    in_names.extend(out_names)
    if partition_name is not None:
        in_names.append(partition_name)
    donate = tuple(range(n_params, n_params + n_outs))

    def _body(*args):
        operands = list(args)
        if partition_name is not None:
            operands.append(b2j.partition_id_tensor())
        outs = b2j._bass_exec_p.bind(
            *operands,
            out_avals=tuple(out_avals),
            in_names=tuple(in_names),
            out_names=tuple(out_names),
            lowering_input_output_aliases=(),
            sim_require_finite=True,
            sim_require_nnan=True,
            nc=nc,
        )
        return tuple(outs)

    devices = jax.devices()[:n_cores]
    mesh = Mesh(np.asarray(devices), ("core",))
    in_specs = (PartitionSpec("core"),) * (n_params + n_outs)
    out_specs = (PartitionSpec("core"),) * len(out_names)
    sharded = jax.jit(
        shard_map(
            _body, mesh=mesh, in_specs=in_specs, out_specs=out_specs, check_rep=False
        ),
        donate_argnums=donate,
        keep_unused=True,
    )
    sh = NamedSharding(mesh, PartitionSpec("core"))
    concat_in = [
        jax.device_put(
            np.concatenate([np.asarray(m[name]) for m in in_maps], axis=0), sh
        )
        for name in in_names[:n_params]
    ]
    concat_zero = [
        np.zeros((n_cores * z.shape[0], *z.shape[1:]), z.dtype) for z in zero_outs
    ]

    def run_batch(iters):
        zs = [[jax.device_put(z, sh) for z in concat_zero] for _ in range(iters)]
        for z in zs:
            for a in z:
                a.block_until_ready()
        t0 = time.perf_counter()
        outs = None
        for i in range(iters):
            outs = sharded(*concat_in, *zs[i])
        for o in outs:
            o.block_until_ready()
        return time.perf_counter() - t0

    run_batch(2)  # warm-up (compile + pipeline)
    marginals = []
    for _ in range(5):
        t_small = run_batch(3)
        t_large = run_batch(13)
        marginals.append((t_large - t_small) / 10.0)
    marginals.sort()
    return int(marginals[len(marginals) // 2] * 1e9)


# revision 56
# speedup vs baseline: 1.4733x; 1.4733x over previous
"""KMeans vq_codebook kernel for 8 trn2 NeuronCores.

Data-parallel over N (per sharding hint), 32768 rows/core, 256 row-tiles of
128. Per 4-tile PSUM batch:

  PE:   bias matmuls: g2 = -csq broadcast (fp16 [2 x 512]-col, PSUM init)
        main matmuls: g2[:,q,:] += xh_tile^T @ ch  (ONE fp16 matmul per tile;
        host splits x to fp16 - dropped lo terms cost ~3e-4 rel acc)
        conf matmuls: fp8 DoubleRow, contracting TWO tiles per instruction
        (yoh padded to 16 classes; indicator values exact in fp8)
  DVE:  hmax = tensor_reduce(max) over the fp32 PSUM batch (exact)
  Indicator (by batch, balancing DVE vs ACT):
    - 7 of 8 batches on ACT: sign(g2 - hmax + DELTA) per tile (per-partition
      bias AP), values {-1,+1} -> confb; host linearizes A=(confb+n_c)/2
    - 1 of 8 batches on DVE: is_ge(g2, hmax) batched (exact) -> confa

  loss = sum(x^2) (host fp64) - sum(hmax);  acc from conf on host.
"""

import numpy as np

try:
    import concourse.bass as bass
    import concourse.mybir as mybir
    import concourse.tile as tile
    from concourse.bass_utils import run_bass_kernel_spmd
except ImportError:  # allow sys.path setup by the caller
    import sys

    sys.path.insert(0, "/opt/trn_rl_repo")
    import concourse.bass as bass
    import concourse.mybir as mybir
    import concourse.tile as tile
    from concourse.bass_utils import run_bass_kernel_spmd

N_FULL = 262144
D = 128
K = 256
NUM_CORES = 8
NS = N_FULL // NUM_CORES  # 32768 rows per core
NTILES = NS // 128  # 256
NUM_GT_CLASSES = 10
NGC = 16  # padded to 16 for fp8 DoubleRow conf matmuls (rows 10..15 zero)

BATCH = 4  # tiles per PSUM batch
BATCH_PERIOD = 7  # every BATCH_PERIOD-th batch runs its indicator on DVE.
# HW-measured optimum: bp=4 -> 245us, bp=7 -> 171us, all-ACT -> 192us; the
# cost model is near-flat here, so trust the hardware A/B.
DELTA = 2e-3  # ACT sign margin: sign(h - hmax + DELTA) is +1 at the argmax

F32 = mybir.dt.float32
F16 = mybir.dt.float16
F8 = mybir.dt.float8e4

_CACHE = {}
LAST = None  # BassKernelResults of the most recent run
HW_EXEC_NS = None  # per-execution device time measured by _bench_exec


def _is_dve_batch(b):
    return (b % BATCH_PERIOD) == (BATCH_PERIOD - 1)


def _is_act_tile(j):
    return not _is_dve_batch(j // BATCH)


LAG_OVERRIDE = 2
OH_BUFS = 5


def build_nc(ns=NS, supertile=2048, batch=BATCH, g2bufs=3, repeats=1):
    """repeats>1 builds a benchmarking variant: the whole per-core program
    body is repeated in-NEFF (identical results, overwritten) so device time
    dominates per-dispatch overhead when measuring."""
    ntiles = ns // 128
    tiles_per_super = supertile // 128
    nbatch = ntiles // batch
    assert tiles_per_super % batch == 0
    assert batch % 2 == 0

    import concourse.bacc as bacc

    nc = bacc.Bacc("TRN2", target_bir_lowering=False, debug=False)

    xh_d = nc.declare_dram_parameter("xh", [D, ns], F16, isOutput=False)
    ch_d = nc.declare_dram_parameter("ch", [D, K], F16, isOutput=False)
    # [2, batch*K] fp16: rows = csq hi/lo, tiled `batch` times, negated
    ncsq_d = nc.declare_dram_parameter("ncsq", [2, batch * K], F16, isOutput=False)
    yoh_d = nc.declare_dram_parameter("yoh", [128, ntiles, NGC], F8, isOutput=False)
    hst_out = nc.declare_dram_parameter("hst", [128, ntiles], F32, isOutput=True)
    confa_out = nc.declare_dram_parameter("confa", [NGC, K], F32, isOutput=True)
    confb_out = nc.declare_dram_parameter("confb", [NGC, K], F32, isOutput=True)

    dve_batches = [b for b in range(nbatch) if _is_dve_batch(b)]
    dve_pairs = [
        (b, p) for b in range(nbatch) for p in range(batch // 2) if _is_dve_batch(b)
    ]
    act_pairs = [
        (b, p)
        for b in range(nbatch)
        for p in range(batch // 2)
        if not _is_dve_batch(b)
    ]
    LAG = LAG_OVERRIDE  # conf matmuls emitted after fill of batch b+LAG

    with tile.TileContext(nc) as tc:
        with (
            tc.tile_pool(name="const", bufs=1) as constp,
            tc.tile_pool(name="xs", bufs=3) as xsp,
            tc.tile_pool(name="oh", bufs=OH_BUFS) as ohp,
            tc.tile_pool(name="hn", bufs=3) as hnp,
            tc.tile_pool(name="acc", bufs=1) as accp,
            tc.tile_pool(name="ps", bufs=g2bufs, space=bass.MemorySpace.PSUM) as psp,
            tc.tile_pool(name="psca", bufs=1, space=bass.MemorySpace.PSUM) as pscap,
            tc.tile_pool(name="pscb", bufs=1, space=bass.MemorySpace.PSUM) as pscbp,
        ):
            ch_t = constp.tile([D, K], F16, tag="ch")
            ones2_t = constp.tile([2, 128], F16, tag="ones2")
            ncsq_t = constp.tile([2, batch * K], F16, tag="ncsq")
            yoh_t = constp.tile([128, ntiles, NGC], F8, tag="yoh")
            nc.sync.dma_start(ch_t[:], ch_d[:, :])
            nc.sync.dma_start(ncsq_t[:], ncsq_d[:, :])
            nc.vector.memset(ones2_t[:], 1.0)

            hstore = accp.tile([128, ntiles], F32, tag="hst")
            confa_ps = pscap.tile([NGC, K], F32, tag="confa")
            confb_ps = pscbp.tile([NGC, K], F32, tag="confb")

            xh_tiles = {}
            n_super = ns // supertile
            PF = 2  # supertiles of x prefetched beyond the one in use

            def issue_xh(st):
                if st in xh_tiles or st >= n_super:
                    return
                xh_s = xsp.tile([D, supertile], F16, tag="xh")
                nc.sync.dma_start(
                    xh_s[:], xh_d[:, st * supertile : (st + 1) * supertile]
                )
                xh_tiles[st] = xh_s

            def emit_fill(b):
                st = b // (tiles_per_super // batch)
                bb = b % (tiles_per_super // batch)
                if bb == 0:
                    issue_xh(st + PF)
                xh_s = xh_tiles[st]
                g2 = psp.tile([128, batch, K], F32, tag="g2")
                g2flat = g2[:].rearrange("p a b -> p (a b)")
                half = batch * K // 2  # 512 fp32 = one PSUM bank
                for hh in range(2):
                    nc.tensor.matmul(
                        g2flat[:, hh * half : (hh + 1) * half],
                        ones2_t[:],
                        ncsq_t[:, hh * half : (hh + 1) * half],
                        start=True,
                        stop=False,
                        skip_group_check=True,
                    )
                for q in range(batch):
                    sl = slice((bb * batch + q) * 128, (bb * batch + q + 1) * 128)
                    nc.tensor.matmul(
                        g2[:, q, :],
                        xh_s[:, sl],
                        ch_t[:],
                        start=False,
                        stop=True,
                        skip_group_check=True,
                    )
                return g2

            def emit_reduce(b, g2):
                """DVE TR-max. DVE batches store +hmax; ACT batches store
                -hmax (used directly as the ACT sign bias; host negates)."""
                jb = b * batch
                nc.vector.tensor_reduce(
                    hstore[:, jb : jb + batch],
                    g2[:],
                    axis=mybir.AxisListType.X,
                    op=mybir.AluOpType.max,
                    negate=not _is_dve_batch(b),
                )

            def emit_signs(b, g2, oh4):
                jb = b * batch
                for q in range(batch):
                    j = jb + q
                    nc.scalar.activation(
                        oh4[:, q, :],
                        g2[:, q, :],
                        mybir.ActivationFunctionType.Sign,
                        bias=hstore[:, j : j + 1],
                        scale=1.0,
                    )

            def emit_iseq(b, g2, oh4):
                """Batched exact compare on DVE (deferred one step so the next
                batch's TR is never stuck behind it)."""
                jb = b * batch
                nc.vector.tensor_tensor(
                    oh4[:],
                    g2[:],
                    hstore[:, jb : jb + batch]
                    .unsqueeze(2)
                    .broadcast_to([128, batch, K]),
                    mybir.AluOpType.is_ge,
                )

            def emit_conf(b, oh4):
                jb = b * batch
                conf_ps = confa_ps if _is_dve_batch(b) else confb_ps
                plist = dve_pairs if _is_dve_batch(b) else act_pairs
                for p in range(batch // 2):
                    j0 = jb + 2 * p
                    nc.tensor.matmul(
                        conf_ps[:],
                        yoh_t[:, j0 : j0 + 2, :],
                        oh4[:, 2 * p : 2 * p + 2, :],
                        start=((b, p) == plist[0]),
                        stop=((b, p) == plist[-1]),
                        skip_group_check=True,
                        perf_mode=mybir.MatmulPerfMode.DoubleRow,
                    )

            oh_tiles = {}
            for _rep in range(repeats):
                xh_tiles.clear()
                for s in range(min(PF + 1, n_super)):
                    issue_xh(s)
                if _rep == 0:
                    # yoh is first needed by the conf stage ~3 batches in;
                    # keep it off the critical startup DMA path
                    nc.sync.dma_start(yoh_t[:], yoh_d[:, :, :])
                g2_tiles = {}
                for t in range(nbatch + LAG + 1):
                    if t < nbatch:
                        g2_tiles[t] = emit_fill(t)
                    b = t - 1
                    if 0 <= b < nbatch:
                        emit_reduce(b, g2_tiles[b])
                        oh4 = ohp.tile([128, batch, K], F8, tag="oh4")
                        oh_tiles[b] = oh4
                        if not _is_dve_batch(b):
                            emit_signs(b, g2_tiles.pop(b), oh4)
                    b = t - 2
                    if 0 <= b < nbatch and _is_dve_batch(b):
                        emit_iseq(b, g2_tiles.pop(b), oh_tiles[b])
                    b = t - LAG - 1
                    if 0 <= b < nbatch:
                        emit_conf(b, oh_tiles.pop(b))

            confa_sb = accp.tile([NGC, K], F32, tag="confasb")
            confb_sb = accp.tile([NGC, K], F32, tag="confbsb")
            if dve_pairs:
                nc.vector.tensor_copy(confa_sb[:], confa_ps[:])
            else:
                nc.vector.memset(confa_sb[:], 0.0)
            if act_pairs:
                nc.vector.tensor_copy(confb_sb[:], confb_ps[:])
            else:
                nc.vector.memset(confb_sb[:], 0.0)
            nc.sync.dma_start(hst_out[:, :], hstore[:])
            nc.sync.dma_start(confa_out[:, :], confa_sb[:])
            nc.sync.dma_start(confb_out[:, :], confb_sb[:])

    nc.compile()
    return nc


def _host_prep(x, y_np, centers):
    """Build per-core input maps from full inputs."""
    import ml_dtypes

    f8 = ml_dtypes.float8_e4m3fn
    xt = np.ascontiguousarray(x.T).astype(np.float16)  # [128, N]
    c2t = np.ascontiguousarray(centers.T) * np.float32(2.0)
    ch = c2t.astype(np.float16)  # [128, K]

    csq = np.sum(centers.astype(np.float64) ** 2, axis=1)
    ncsq_hi = (-csq).astype(np.float16)
    ncsq_lo = ((-csq) - ncsq_hi.astype(np.float64)).astype(np.float16)
    ncsq = np.empty((2, BATCH * K), np.float16)
    ncsq[0] = np.tile(ncsq_hi, BATCH)
    ncsq[1] = np.tile(ncsq_lo, BATCH)

    y_cores = y_np.reshape(NUM_CORES, NTILES, 128)  # [core, tile, p]
    oh = (y_cores[:, :, :, None] == np.arange(NGC)[None, None, None, :]).astype(f8)
    yoh_all = np.ascontiguousarray(oh.transpose(0, 2, 1, 3))  # [core,128,tile,16]

    in_maps = []
    for c in range(NUM_CORES):
        sl = slice(c * NS, (c + 1) * NS)
        in_maps.append(
            {
                "xh": np.ascontiguousarray(xt[:, sl]),
                "ch": ch,
                "ncsq": ncsq,
                "yoh": yoh_all[c],
            }
        )
    return in_maps


def kernel(x, y, centers):
    x = np.asarray(x, dtype=np.float32)
    y_np = np.asarray(y).astype(np.int64)
    centers = np.asarray(centers, dtype=np.float32)
    n = x.shape[0]
    assert n == N_FULL and x.shape[1] == D and centers.shape == (K, D)

    if "nc" not in _CACHE:
        _CACHE["nc"] = build_nc()
    nc = _CACHE["nc"]

    in_maps = _host_prep(x, y_np, centers)

    kr = run_bass_kernel_spmd(nc, in_maps, list(range(NUM_CORES)))
    global LAST, HW_EXEC_NS
    LAST = kr
    res = kr.results

    import os

    if os.environ.get("BASS_BENCH") == "1":
        reps = int(os.environ.get("BASS_BENCH_REPS", "16"))
        if reps > 1:
            if "nc_bench" not in _CACHE:
                _CACHE["nc_bench"] = build_nc(repeats=reps)
            HW_EXEC_NS = _bench_exec(_CACHE["nc_bench"], in_maps, NUM_CORES) // reps
        else:
            HW_EXEC_NS = _bench_exec(nc, in_maps, NUM_CORES)

    # ACT-routed tiles store (DELTA - hmax) and count into confb with sign
    # values {-1,+1} (linearized below); DVE tiles store +hmax -> confa.
    act_tile = np.array([_is_act_tile(j) for j in range(NTILES)])

    hmax_sum = 0.0
    confa = np.zeros((NGC, K), np.float64)
    confb = np.zeros((NGC, K), np.float64)
    for c in range(NUM_CORES):
        hst = np.asarray(res[c]["hst"]).astype(np.float64)  # [128, ntiles]
        hmax = np.where(act_tile[None, :], -hst, hst)
        hmax_sum += hmax.sum()
        confa += np.asarray(res[c]["confa"]).astype(np.float64)
        confb += np.asarray(res[c]["confb"]).astype(np.float64)

    # per-class sample counts within ACT tiles (same tile pattern per core)
    y_tiles = y_np.reshape(NUM_CORES * NTILES, 128)
    act_mask = np.tile(act_tile, NUM_CORES)
    n_act = np.bincount(y_tiles[act_mask].ravel(), minlength=NGC).astype(np.float64)

    # ACT path computed sign(h - hmax): argmax element -> sign(0) = s0 (HW
    # table value, 0 or 1), others -> -1. confb[c,k] = (s0+1)*A - n_c, so
    # A = (confb + n_c) / (s0 + 1). Calibrate s0 from the column sums:
    # sum_k confb[c,:] = (s0+1)*n_c - K*n_c.
    nz = n_act > 0
    s0_est = (confb.sum(axis=1)[nz] / n_act[nz]).mean() + K - 1
    s0 = int(round(s0_est))
    assert s0 in (0, 1), f"unexpected HW sign(0) semantics: s0={s0_est}"
    conf = confa + (confb + n_act[:, None]) / (s0 + 1.0)  # [16, K]

    x64 = x.astype(np.float64)
    x_sq_total = float(np.einsum("nd,nd->", x64, x64, optimize=True))
    loss = np.float32(x_sq_total - hmax_sum)

    correct_ct = conf[:NUM_GT_CLASSES].max(axis=0).sum()
    acc = np.float32(correct_ct / np.float64(n))
    return loss, acc


def _bench_exec(nc, in_maps, n_cores):
    """Estimate per-execution device time of the compiled NEFF.

    Replicates bass2jax.run_bass_via_pjrt's jit(shard_map(custom_call))
    plumbing, but keeps the jitted callable and times pipelined repeated
    executions, reporting the marginal time per execution.
    """
    import time

    import jax
    from jax.experimental.shard_map import shard_map
    from jax.sharding import Mesh, NamedSharding, PartitionSpec

    from concourse import bass2jax as b2j

    b2j.install_neuronx_cc_hook()
    partition_name = nc.partition_id_tensor.name if nc.partition_id_tensor else None
    in_names, out_names, out_avals, zero_outs = [], [], [], []
    for alloc in nc.m.functions[0].allocations:
        if not isinstance(alloc, mybir.MemoryLocationSet):
            continue
        name = alloc.memorylocations[0].name
        if alloc.kind == "ExternalInput":
            if name != partition_name:
                in_names.append(name)
        elif alloc.kind == "ExternalOutput":
            out_names.append(name)
            shape = tuple(alloc.tensor_shape)
            dtype = mybir.dt.np(alloc.dtype)
            out_avals.append(jax.core.ShapedArray(shape, dtype))
            zero_outs.append(np.zeros(shape, dtype))
    n_params = len(in_names)
    n_outs = len(out_avals)
    in_names.extend(out_names)
    if partition_name is not None:
        in_names.append(partition_name)
    donate = tuple(range(n_params, n_params + n_outs))

    def _body(*args):
        operands = list(args)
        if partition_name is not None:
            operands.append(b2j.partition_id_tensor())
        outs = b2j._bass_exec_p.bind(
            *operands,
            out_avals=tuple(out_avals),
            in_names=tuple(in_names),
            out_names=tuple(out_names),
            lowering_input_output_aliases=(),
            sim_require_finite=True,
            sim_require_nnan=True,
            nc=nc,
        )
        return tuple(outs)

    devices = jax.devices()[:n_cores]
    mesh = Mesh(np.asarray(devices), ("core",))
    in_specs = (PartitionSpec("core"),) * (n_params + n_outs)
    out_specs = (PartitionSpec("core"),) * len(out_names)
    sharded = jax.jit(
        shard_map(
            _body, mesh=mesh, in_specs=in_specs, out_specs=out_specs, check_rep=False
        ),
        donate_argnums=donate,
        keep_unused=True,
    )
    sh = NamedSharding(mesh, PartitionSpec("core"))
    concat_in = [
        jax.device_put(
            np.concatenate([np.asarray(m[name]) for m in in_maps], axis=0), sh
        )
        for name in in_names[:n_params]
    ]
    concat_zero = [
        np.zeros((n_cores * z.shape[0], *z.shape[1:]), z.dtype) for z in zero_outs
    ]

    def run_batch(iters):
        zs = [[jax.device_put(z, sh) for z in concat_zero] for _ in range(iters)]
        for z in zs:
            for a in z:
                a.block_until_ready()
        t0 = time.perf_counter()
        outs = None
        for i in range(iters):
            outs = sharded(*concat_in, *zs[i])
        for o in outs:
            o.block_until_ready()
        return time.perf_counter() - t0

    run_batch(2)  # warm-up (compile + pipeline)
    marginals = []
    for _ in range(5):
        t_small = run_batch(3)
        t_large = run_batch(13)
        marginals.append((t_large - t_small) / 10.0)
    marginals.sort()
    return int(marginals[len(marginals) // 2] * 1e9)
